# revision 1
# baseline (speedup 1.0000x reference)
"""MinkowskiResBlock on 8 TRN2 NeuronCores.

Strategy: spatially shard the N=131072 points across 8 cores (coords are
reconstructed from the labeled 27-offset neighbor graph), per-core local
gather tables (own shard + halo) in HBM, transposed dma_gather feeding bf16
matmuls that accumulate out^T in PSUM, BN stats via ACT accum + AllReduce,
halo exchange of the intermediate activations via export-gather + AllGather
+ parity import-gathers.  Falls back to a NumPy reference path if the
neighbor graph is not grid-consistent or a shard overflows its caps.
"""

import numpy as np
import ml_dtypes

N, C, K, NCORES = 131072, 192, 27, 8
S = N // NCORES            # 16384 points per core
ELEM = 256                 # bf16 elems per table row (C=192 + 64 pad) = 512B
RT = 512                   # rowtile (gather size / PSUM free dim)
NT = S // RT               # 32 rowtiles per core
HALO_CAP = 8192
EXP_CAP = 8192             # per-core export slots (8 * 8192 = 65536 rows)
IMP_CAP = 4096             # per parity class
ZROW1 = S + HALO_CAP       # T1 zero row
L1 = ZROW1 + 1
ZROW2 = S + 2 * IMP_CAP    # T2 zero row
L2 = ZROW2 + 1
BN_EPS = 1e-5

OFFS = np.array(
    [[dx, dy, dz] for dx in (-1, 0, 1) for dy in (-1, 0, 1) for dz in (-1, 0, 1)],
    np.int64,
)

_PROGRAM_CACHE = {}


# ----------------------------------------------------------------------------
# host-side graph analysis / sharding
# ----------------------------------------------------------------------------

def _spatial_order(neigh):
    """Reconstruct voxel coords from the labeled neighbor graph; return a
    spatial ordering of the N points, or None if the graph is inconsistent."""
    nb_all = neigh.astype(np.int64)
    if nb_all.shape != (K, N) or nb_all.min() < 0 or nb_all.max() > N:
        return None
    coords = np.zeros((N, 3), np.int64)
    comp = np.full(N, -1, np.int64)
    visited = np.zeros(N, bool)
    ncomp = 0
    while True:
        seeds = np.flatnonzero(~visited)
        if seeds.size == 0:
            break
        seed = seeds[0]
        visited[seed] = True
        comp[seed] = ncomp
        frontier = np.array([seed], np.int64)
        while frontier.size:
            new = []
            for k in range(K):
                if k == 13:
                    continue
                nb = nb_all[k][frontier]
                valid = nb < N
                if not valid.any():
                    continue
                src = frontier[valid]
                dst = nb[valid]
                fresh = ~visited[dst]
                if fresh.any():
                    d = dst[fresh]
                    s = src[fresh]
                    coords[d] = coords[s] + OFFS[k]
                    visited[d] = True
                    comp[d] = ncomp
                    new.append(d)
            frontier = (
                np.unique(np.concatenate(new)) if new else np.array([], np.int64)
            )
        ncomp += 1
        if ncomp > 64:  # clearly not a sparse voxel grid
            return None
    # validate every edge against its labeled offset
    for k in range(K):
        if k == 13:
            continue
        nb = nb_all[k]
        valid = np.flatnonzero(nb < N)
        if valid.size == 0:
            continue
        dst = nb[valid]
        if not (comp[dst] == comp[valid]).all():
            return None
        if not (coords[dst] == coords[valid] + OFFS[k]).all():
            return None
    if not (nb_all[13][nb_all[13] < N] == np.flatnonzero(nb_all[13] < N)).all():
        # self-offset must map each point to itself where valid
        pass  # not strictly required for correctness of our scheme
    key = coords - coords.min(axis=0)
    return np.lexsort((key[:, 2], key[:, 1], key[:, 0], comp))


def _pack_idx(flat):
    """int16 [n] (n % 16 == 0) -> [128, n//16] in the firmware layout:
    idx j at partition j%16, offset j//16, replicated across the 8
    16-partition groups (each SWDGE queue's Q7 pair reads its own group)."""
    lay = flat.reshape(-1, 16).T.astype(np.int16)  # [16, n//16]
    return np.tile(lay, (8, 1))


def _prepare_host(feats, W1, gamma1, beta1, W2, gamma2, beta2, neigh):
    order = _spatial_order(neigh)
    if order is None:
        return None
    nb = neigh.astype(np.int64)
    owner = np.empty(N, np.int64)
    ownpos = np.empty(N, np.int64)
    for c in range(NCORES):
        ids = order[c * S:(c + 1) * S]
        owner[ids] = c
        ownpos[ids] = np.arange(S)

    own_ids, halos = [], []
    for c in range(NCORES):
        ids = order[c * S:(c + 1) * S]
        own_ids.append(ids)
        fan = nb[:, ids].ravel()
        fan = np.unique(fan[fan < N])
        halo = fan[owner[fan] != c]
        if halo.size > HALO_CAP:
            return None
        halos.append(halo)

    # per-source export lists: union of halo rows each core must serve
    exp_ids = [[] for _ in range(NCORES)]
    for c in range(NCORES):
        for s, cnt in zip(*np.unique(owner[halos[c]], return_counts=True)):
            exp_ids[int(s)].append(halos[c][owner[halos[c]] == s])
    exports = []
    pos = np.full(N, -1, np.int64)  # global export-table position per id
    for s in range(NCORES):
        e = (
            np.unique(np.concatenate(exp_ids[s]))
            if exp_ids[s]
            else np.array([], np.int64)
        )
        if e.size > EXP_CAP:
            return None
        exports.append(e)
        pos[e] = s * EXP_CAP + np.arange(e.size)

    feats_bf = np.zeros((N + 1, ELEM), ml_dtypes.bfloat16)
    feats_bf[:N, :C] = feats.astype(ml_dtypes.bfloat16)

    w_scale = 1.0  # weights used as-is
    def pack_w(W):
        wp = np.zeros((K, 2, 128, C), ml_dtypes.bfloat16)
        for k in range(K):
            wp[k, 0, :, :] = (W[k][0:128, :] * w_scale).astype(ml_dtypes.bfloat16)
            wp[k, 1, 0:64, :] = (W[k][128:192, :] * w_scale).astype(ml_dtypes.bfloat16)
        return wp

    w1p, w2p = pack_w(W1), pack_w(W2)

    gb = np.zeros((128, 8), np.float32)
    gb[:, 0] = gamma1[0:128]
    gb[0:64, 1] = gamma1[128:192]
    gb[:, 2] = beta1[0:128]
    gb[0:64, 3] = beta1[128:192]
    gb[:, 4] = gamma2[0:128]
    gb[0:64, 5] = gamma2[128:192]
    gb[:, 6] = beta2[0:128]
    gb[0:64, 7] = beta2[128:192]

    eye16 = np.eye(128, dtype=ml_dtypes.bfloat16)
    eye32 = np.eye(128, dtype=np.float32)

    in_maps = []
    for c in range(NCORES):
        ids = own_ids[c]
        halo = halos[c]

        # T1: [own | halo(sorted) | zero]
        t1 = np.zeros((L1, ELEM), ml_dtypes.bfloat16)
        t1[0:S] = feats_bf[ids]
        t1[S:S + halo.size] = feats_bf[halo]
        loc1 = np.full(N + 1, ZROW1, np.int64)
        loc1[ids] = np.arange(S)
        loc1[halo] = S + np.arange(halo.size)

        # T2 layout: [own | imp_even | imp_odd | zero]
        hpos = pos[halo]
        assert (hpos >= 0).all()
        even_m = (hpos & 1) == 0
        he, ho = halo[even_m], halo[~even_m]
        if he.size > IMP_CAP or ho.size > IMP_CAP:
            return None
        loc2 = np.full(N + 1, ZROW2, np.int64)
        loc2[ids] = np.arange(S)
        loc2[he] = S + np.arange(he.size)
        loc2[ho] = S + IMP_CAP + np.arange(ho.size)

        nbo = nb[:, ids]  # [27, S] global
        idx1 = loc1[nbo]  # [27, S]
        idx2 = loc2[nbo]
        assert idx1.max() < 32768 and idx2.max() < 32768

        idx1_dev = np.zeros((128, K, NT, RT // 16), np.int16)
        idx2_dev = np.zeros((128, K, NT, RT // 16), np.int16)
        for k in range(K):
            for t in range(NT):
                idx1_dev[:, k, t, :] = _pack_idx(idx1[k, t * RT:(t + 1) * RT])
                idx2_dev[:, k, t, :] = _pack_idx(idx2[k, t * RT:(t + 1) * RT])

        # export gather: local own rows to publish (padded with 0)
        eloc = np.zeros(EXP_CAP, np.int64)
        eloc[:exports[c].size] = ownpos[exports[c]]
        exp_dev = np.zeros((128, EXP_CAP // RT, RT // 16), np.int16)
        for gidx in range(EXP_CAP // RT):
            exp_dev[:, gidx, :] = _pack_idx(eloc[gidx * RT:(gidx + 1) * RT])

        # import gathers: ag positions >> 1 per parity class (padded with 0)
        imp_dev = np.zeros((128, 2, IMP_CAP // RT, RT // 16), np.int16)
        for pcls, h in enumerate((he, ho)):
            ip = np.zeros(IMP_CAP, np.int64)
            ip[:h.size] = pos[h] >> 1
            for gidx in range(IMP_CAP // RT):
                imp_dev[:, pcls, gidx, :] = _pack_idx(ip[gidx * RT:(gidx + 1) * RT])

        featsT = np.zeros((2, 128, S), np.float32)
        fo = feats[ids]  # [S, C] fp32
        featsT[0] = fo[:, 0:128].T
        featsT[1, 0:64] = fo[:, 128:192].T

        in_maps.append({
            "t1": t1,
            "idx1": idx1_dev,
            "idx2": idx2_dev,
            "expidx": exp_dev,
            "impidx": imp_dev,
            "w1": w1p,
            "w2": w2p,
            "gb": gb,
            "eye16": eye16,
            "eye32": eye32,
            "featsT": featsT,
        })
    return in_maps, own_ids


# ----------------------------------------------------------------------------
# device program
# ----------------------------------------------------------------------------

def _build_program(variant="full"):
    import concourse.tile as tile
    from concourse import bacc, mybir

    AF = mybir.ActivationFunctionType
    ALU = mybir.AluOpType
    bf16, f32, i16 = mybir.dt.bfloat16, mybir.dt.float32, mybir.dt.int16

    nc = bacc.Bacc("TRN2", target_bir_lowering=False, debug=False,
                   num_devices=NCORES, num_swdge_queues=4)
    if variant == "floor":
        out_f = nc.dram_tensor("out", [S, C], f32, kind="ExternalOutput")
        with tile.TileContext(nc) as tc:
            with tc.tile_pool(name="fl", bufs=1) as flp:
                z = flp.tile([128, 16, C], f32, tag="z")
                nc.vector.memset(z[:], 0.0)
                for j in range(8):
                    nc.sync.dma_start(
                        out_f[j * 2048:(j + 1) * 2048].rearrange(
                            "(t p) e -> p t e", p=128), z[:])
        nc.compile()
        return nc

    t1_d = nc.dram_tensor("t1", [L1, ELEM], bf16, kind="ExternalInput")
    idx1_d = nc.dram_tensor("idx1", [128, K, NT, RT // 16], i16, kind="ExternalInput")
    idx2_d = nc.dram_tensor("idx2", [128, K, NT, RT // 16], i16, kind="ExternalInput")
    exp_d = nc.dram_tensor("expidx", [128, EXP_CAP // RT, RT // 16], i16, kind="ExternalInput")
    imp_d = nc.dram_tensor("impidx", [128, 2, IMP_CAP // RT, RT // 16], i16, kind="ExternalInput")
    w1_d = nc.dram_tensor("w1", [K, 2, 128, C], bf16, kind="ExternalInput")
    w2_d = nc.dram_tensor("w2", [K, 2, 128, C], bf16, kind="ExternalInput")
    gb_d = nc.dram_tensor("gb", [128, 8], f32, kind="ExternalInput")
    eye16_d = nc.dram_tensor("eye16", [128, 128], bf16, kind="ExternalInput")
    eye32_d = nc.dram_tensor("eye32", [128, 128], f32, kind="ExternalInput")
    fT_d = nc.dram_tensor("featsT", [2, 128, S], f32, kind="ExternalInput")
    out_d = nc.dram_tensor("out", [S, C], f32, kind="ExternalOutput")

    with tile.TileContext(nc) as tc:
        with (
            tc.tile_pool(name="const", bufs=1) as constp,
            tc.tile_pool(name="widx", bufs=1) as widxp,
            tc.tile_pool(name="idxs", bufs=2) as idxp,
            tc.tile_pool(name="big", bufs=1) as bigp,
            tc.tile_pool(name="gat", bufs=6) as gatp,
            tc.tile_pool(name="work", bufs=2) as workp,
            tc.tile_pool(name="stat", bufs=1) as statp,
            tc.tile_pool(name="pacc", bufs=2, space="PSUM") as paccp,
            tc.tile_pool(name="ptr", bufs=2, space="PSUM") as ptrp,
            tc.tile_pool(name="dram", bufs=1, space="DRAM") as dramp,
        ):
            # ---------------- constants ----------------
            gb_t = constp.tile([128, 8], f32)
            nc.sync.dma_start(gb_t[:], gb_d[:])
            eye16_t = constp.tile([128, 128], bf16)
            nc.sync.dma_start(eye16_t[:], eye16_d[:])
            eye32_t = constp.tile([128, 128], f32)
            nc.sync.dma_start(eye32_t[:], eye32_d[:])
            expidx_t = constp.tile([128, EXP_CAP // RT, RT // 16], i16)
            nc.sync.dma_start(expidx_t[:], exp_d[:])
            impidx_t = constp.tile([128, 2, IMP_CAP // RT, RT // 16], i16)
            nc.sync.dma_start(impidx_t[:], imp_d[:])

            # internal DRAM
            t2_t = dramp.tile([L2, ELEM], bf16)
            expbuf_t = dramp.tile([EXP_CAP, ELEM], bf16)
            ag_t = dramp.tile([NCORES * EXP_CAP, ELEM], bf16, addr_space="Shared")
            ar_in = [dramp.tile([128, 4], f32, name=f"ar_in{i}") for i in range(2)]
            ar_out = [dramp.tile([128, 4], f32, addr_space="Shared", name=f"ar_out{i}")
                      for i in range(2)]

            # persistent per-conv state (tags shared between convs)
            o_a = [None]
            o_b = [None]

            do_gather = variant != "mm"
            do_mm = variant not in ("gath", "gath4q")
            gq = (lambda k: k % 4) if variant == "gath4q" else (lambda k: 0)

            def conv(conv_i, table_ap, idx_dram, w_tile):
                """Gather-GEMM over 27 offsets; returns nothing, fills
                o_a/o_b (bf16 out^T halves) and stats tiles."""
                sums = statp.tile([128, NT, 2], f32, tag=f"sums{conv_i}")
                sums_b = statp.tile([64, NT, 2], f32, tag=f"sumsb{conv_i}")
                oa = bigp.tile([128, S], bf16, tag="o_a")
                ob = bigp.tile([64, S], bf16, tag="o_b")
                if not do_mm:
                    nc.vector.memset(sums[:], 0.0)
                    nc.vector.memset(sums_b[:], 0.0)
                    nc.vector.memset(oa[:], 0.0)
                    nc.vector.memset(ob[:], 0.0)
                o_a[0], o_b[0] = oa, ob
                IG = 4  # rowtiles per idx load
                for t in range(NT):
                    if t % IG == 0:
                        idxg = idxp.tile([128, K, IG, RT // 16], i16, tag="idxg")
                        nc.sync.dma_start(idxg[:], idx_dram[:, :, t:t + IG, :])
                    p1 = paccp.tile([128, RT], f32, tag="p1")
                    p2 = paccp.tile([64, RT], f32, tag="p2")
                    for k in range(K):
                        if do_gather or (k == 0 and t == 0):
                            g = gatp.tile([128, 2, RT], bf16, tag="g")
                            nc.gpsimd.dma_gather(
                                g[:], table_ap, idxg[:, k, t % IG, :], RT, RT, ELEM,
                                transpose=True, queue_num=gq(k),
                            )
                            glast = g
                        else:
                            g = glast
                        if do_mm:
                            for r in range(2):
                                nc.tensor.matmul(
                                    p1[:], w_tile[:, k, r, 0:128], g[:, r, :],
                                    start=(k == 0 and r == 0),
                                    stop=(k == K - 1 and r == 1),
                                )
                            for r in range(2):
                                nc.tensor.matmul(
                                    p2[:], w_tile[:, k, r, 128:192], g[:, r, :],
                                    start=(k == 0 and r == 0),
                                    stop=(k == K - 1 and r == 1),
                                )
                    if not do_mm:
                        continue
                    ts = slice(t * RT, (t + 1) * RT)
                    sq1 = workp.tile([128, RT], f32, tag="sq1")
                    sq2 = workp.tile([64, RT], f32, tag="sq2")
                    nc.scalar.activation(oa[:, ts], p1[:], AF.Copy,
                                         accum_out=sums[:, t, 0:1])
                    nc.scalar.activation(sq1[:], p1[:], AF.Square,
                                         accum_out=sums[:, t, 1:2])
                    nc.scalar.activation(ob[:, ts], p2[:], AF.Copy,
                                         accum_out=sums_b[:, t, 0:1])
                    nc.scalar.activation(sq2[:], p2[:], AF.Square,
                                         accum_out=sums_b[:, t, 1:2])
                return sums, sums_b

            def bn_coeffs(conv_i, sums, sums_b):
                """AllReduce the per-rowtile partial sums; compute per-channel
                a = gamma*rsqrt(var+eps), b = beta - mean*a as [128,1]/[64,1]."""
                tot = statp.tile([128, 4], f32, tag=f"tot{conv_i}")
                junk1 = workp.tile([128, NT], f32, tag="sq1")
                junk2 = workp.tile([64, NT], f32, tag="sq2")
                nc.vector.memset(tot[:], 0.0)
                nc.scalar.activation(junk1[:], sums[:, :, 0], AF.Copy,
                                     accum_out=tot[:, 0:1])
                nc.scalar.activation(junk1[:], sums[:, :, 1], AF.Copy,
                                     accum_out=tot[:, 1:2])
                nc.scalar.activation(junk2[:], sums_b[:, :, 0], AF.Copy,
                                     accum_out=tot[0:64, 2:3])
                nc.scalar.activation(junk2[:], sums_b[:, :, 1], AF.Copy,
                                     accum_out=tot[0:64, 3:4])
                nc.sync.dma_start(ar_in[conv_i][:], tot[:])
                rtot = statp.tile([128, 4], f32, tag=f"rtot{conv_i}")
                if variant == "noar":
                    nc.vector.tensor_scalar_mul(rtot[:], tot[:], float(NCORES))
                else:
                    nc.gpsimd.collective_compute(
                        "AllReduce", ALU.add,
                        replica_groups=[list(range(NCORES))],
                        ins=[ar_in[conv_i].opt()], outs=[ar_out[conv_i].opt()],
                    )
                    nc.sync.dma_start(rtot[:], ar_out[conv_i][:])
                # mean/var/a/b per partition, lo (cols 0:2) and hi (cols 2:4)
                co = statp.tile([128, 8], f32, tag=f"co{conv_i}")
                ga = gb_t[:, 4 * conv_i + 0:4 * conv_i + 1]
                ga_h = gb_t[0:64, 4 * conv_i + 1:4 * conv_i + 2]
                be = gb_t[:, 4 * conv_i + 2:4 * conv_i + 3]
                be_h = gb_t[0:64, 4 * conv_i + 3:4 * conv_i + 4]
                invn = 1.0 / float(N)
                for half, (sm, sq, gg, bb) in enumerate((
                    (rtot[:, 0:1], rtot[:, 1:2], ga, be),
                    (rtot[0:64, 2:3], rtot[0:64, 3:4], ga_h, be_h),
                )):
                    p = slice(0, 128) if half == 0 else slice(0, 64)
                    mean = co[p, 4 * half + 0:4 * half + 1]
                    var = co[p, 4 * half + 1:4 * half + 2]
                    a = co[p, 4 * half + 2:4 * half + 3]
                    b = co[p, 4 * half + 3:4 * half + 4]
                    nc.vector.tensor_scalar_mul(mean, sm, invn)
                    nc.vector.tensor_scalar_mul(var, sq, invn)
                    # var -= mean^2 ; var += eps
                    nc.vector.tensor_tensor(a, mean, mean, ALU.mult)
                    nc.vector.tensor_tensor(var, var, a, ALU.subtract)
                    nc.vector.tensor_scalar_add(var, var, BN_EPS)
                    nc.scalar.sqrt(a, var)
                    nc.vector.reciprocal(a, a)          # a = rsqrt(var+eps)
                    nc.vector.tensor_tensor(a, a, gg, ALU.mult)
                    nc.vector.tensor_tensor(b, mean, a, ALU.mult)
                    nc.vector.tensor_tensor(b, bb, b, ALU.subtract)
                return co

            # ======================= conv1 =======================
            w1_t = widxp.tile([128, K, 2, C], bf16, tag="w")
            for k in range(K):
                for r in range(2):
                    nc.sync.dma_start(w1_t[:, k, r, :], w1_d[k, r])

            sums1, sums1_b = conv(0, t1_d[:, :], idx1_d, w1_t)
            if variant == "nobn":
                with tc.tile_pool(name="dumm", bufs=2) as dummp:
                    for t in range(NT):
                        z2 = dummp.tile([128, 4, C], f32, tag="z2")
                        nc.vector.tensor_copy(z2[:, 0, 0:1], sums1[:, 0, 0:1])
                        nc.vector.memset(z2[:, 1:, :], 0.0)
                        nc.sync.dma_start(
                            out_d[t * RT:(t + 1) * RT].rearrange(
                                "(rb p) e -> p rb e", p=128), z2[:])
                nc.compile()
                return nc
            co1 = bn_coeffs(0, sums1, sums1_b)
            a1 = co1[:, 2:3]
            b1 = co1[:, 3:4]
            a1h = co1[0:64, 6:7]
            b1h = co1[0:64, 7:8]

            # BN1 + relu -> h (bf16), transpose to row-major, write T2 own
            oa, ob = o_a[0], o_b[0]
            for t in range(NT):
                ts = slice(t * RT, (t + 1) * RT)
                h1 = workp.tile([128, RT], bf16, tag="h1")
                h2 = workp.tile([64, RT], bf16, tag="h2")
                nc.scalar.activation(h1[:], oa[:, ts], AF.Relu, bias=b1, scale=a1)
                nc.scalar.activation(h2[:], ob[:, ts], AF.Relu, bias=b1h, scale=a1h)
                stage = workp.tile([128, 4, ELEM], bf16, tag="stage")
                nc.vector.memset(stage[:, :, 192:256], 0.0)
                for rb in range(4):
                    tr1 = ptrp.tile([128, 128], bf16, tag="tr1")
                    nc.tensor.transpose(tr1[:], h1[:, rb * 128:(rb + 1) * 128],
                                        eye16_t[:])
                    tr2 = ptrp.tile([128, 64], bf16, tag="tr2")
                    nc.tensor.transpose(tr2[:], h2[:, rb * 128:(rb + 1) * 128],
                                        eye16_t[0:64, 0:64])
                    nc.vector.tensor_copy(stage[:, rb, 0:128], tr1[:])
                    nc.vector.tensor_copy(stage[:, rb, 128:192], tr2[:])
                nc.sync.dma_start(
                    t2_t[ts].rearrange("(rb p) e -> p rb e", p=128), stage[:]
                )
            # T2 zero row
            zrow = constp.tile([1, ELEM], bf16)
            nc.vector.memset(zrow[:], 0.0)
            nc.sync.dma_start(t2_t[ZROW2:ZROW2 + 1, :], zrow[:])

            if variant in ("conv1", "gath", "mm", "gath4q", "noar"):
                for t in range(NT):
                    z = workp.tile([128, 4, C], f32, tag="ostage")
                    nc.vector.memset(z[:], 0.0)
                    nc.sync.dma_start(
                        out_d[t * RT:(t + 1) * RT].rearrange(
                            "(rb p) e -> p rb e", p=128), z[:])
            do_rest = variant == "full"
            if do_rest:
    # ---------------- halo exchange ----------------
                for gidx in range(EXP_CAP // RT):
                    ge = gatp.tile([128, RT // 128, ELEM], bf16, tag="ge")
                    nc.gpsimd.dma_gather(
                        ge[:], t2_t[0:S, :], expidx_t[:, gidx, :], RT, RT, ELEM,
                        transpose=False, queue_num=1 + gidx % 3,
                    )
                    nc.sync.dma_start(
                        expbuf_t[gidx * RT:(gidx + 1) * RT].rearrange(
                            "(t p) e -> p t e", p=128),
                        ge[:],
                    )
                nc.gpsimd.collective_compute(
                    "AllGather", mybir.AluOpType.bypass,
                    replica_groups=[list(range(NCORES))],
                    ins=[expbuf_t.opt()], outs=[ag_t.opt()],
                )
                ag_pairs = ag_t[:].rearrange("(a b) e -> a (b e)", b=2)  # [32768, 512]
                for pcls in range(2):
                    src = ag_pairs[:, pcls * ELEM:(pcls + 1) * ELEM]
                    for gidx in range(IMP_CAP // RT):
                        gi = gatp.tile([128, RT // 128, ELEM], bf16, tag="ge")
                        nc.gpsimd.dma_gather(
                            gi[:], src, impidx_t[:, pcls, gidx, :], RT, RT, ELEM,
                            elem_step=2 * ELEM,
                            transpose=False, queue_num=1 + gidx % 3,
                        )
                        base = S + pcls * IMP_CAP + gidx * RT
                        nc.sync.dma_start(
                            t2_t[base:base + RT].rearrange("(t p) e -> p t e", p=128),
                            gi[:],
                        )

                # ======================= conv2 =======================
                w2_t = widxp.tile([128, K, 2, C], bf16, tag="w")
                for k in range(K):
                    for r in range(2):
                        nc.sync.dma_start(w2_t[:, k, r, :], w2_d[k, r])

                sums2, sums2_b = conv(1, t2_t[:, :], idx2_d, w2_t)
                co2 = bn_coeffs(1, sums2, sums2_b)
                a2 = co2[:, 2:3]
                b2 = co2[:, 3:4]
                a2h = co2[0:64, 6:7]
                b2h = co2[0:64, 7:8]

                # BN2 + residual + relu -> transpose -> out
                oa, ob = o_a[0], o_b[0]
                for t in range(NT):
                    ts = slice(t * RT, (t + 1) * RT)
                    tmp1 = workp.tile([128, RT], f32, tag="tmp1")
                    tmp2 = workp.tile([64, RT], f32, tag="tmp2")
                    nc.vector.tensor_scalar(tmp1[:], oa[:, ts], a2, b2,
                                            ALU.mult, ALU.add)
                    nc.vector.tensor_scalar(tmp2[:], ob[:, ts], a2h, b2h,
                                            ALU.mult, ALU.add)
                    ft1 = workp.tile([128, RT], f32, tag="ft1")
                    nc.sync.dma_start(ft1[:], fT_d[0, :, ts])
                    ft2 = workp.tile([64, RT], f32, tag="ft2")
                    nc.sync.dma_start(ft2[:], fT_d[1, 0:64, ts])
                    nc.vector.tensor_add(tmp1[:], tmp1[:], ft1[:])
                    nc.vector.tensor_add(tmp2[:], tmp2[:], ft2[:])
                    nc.vector.tensor_scalar_max(tmp1[:], tmp1[:], 0.0)
                    nc.vector.tensor_scalar_max(tmp2[:], tmp2[:], 0.0)
                    ostage = workp.tile([128, 4, C], f32, tag="ostage")
                    for rb in range(4):
                        tr1 = ptrp.tile([128, 128], f32, tag="tr1")
                        nc.tensor.transpose(tr1[:], tmp1[:, rb * 128:(rb + 1) * 128],
                                            eye32_t[:])
                        tr2 = ptrp.tile([128, 64], f32, tag="tr2")
                        nc.tensor.transpose(tr2[:], tmp2[:, rb * 128:(rb + 1) * 128],
                                            eye32_t[0:64, 0:64])
                        nc.vector.tensor_copy(ostage[:, rb, 0:128], tr1[:])
                        nc.vector.tensor_copy(ostage[:, rb, 128:192], tr2[:])
                    nc.sync.dma_start(
                        out_d[ts].rearrange("(rb p) e -> p rb e", p=128), ostage[:]
                    )

    nc.compile()
    return nc


# ----------------------------------------------------------------------------
# numpy fallback (also the correctness oracle for arbitrary inputs)
# ----------------------------------------------------------------------------

def _numpy_path(feats, W1, gamma1, beta1, W2, gamma2, beta2, neigh):
    def conv(f, W):
        pad = np.concatenate([f, np.zeros((1, f.shape[1]), f.dtype)], axis=0)
        out = np.zeros_like(f)
        for k in range(W.shape[0]):
            out += pad[neigh[k]] @ W[k]
        return out

    def bn(x, g, b):
        m = x.mean(axis=0)
        v = x.var(axis=0)
        return (x - m) / np.sqrt(v + BN_EPS) * g + b

    out = conv(feats, W1)
    out = np.maximum(bn(out, gamma1, beta1), 0)
    out = conv(out, W2)
    out = bn(out, gamma2, beta2)
    return np.maximum(out + feats, 0).astype(np.float32)


# ----------------------------------------------------------------------------
# entry point
# ----------------------------------------------------------------------------

def _run_device(in_maps, variant="full"):
    from concourse.bass_utils import run_bass_kernel_spmd

    key = f"nc:{variant}"
    if key not in _PROGRAM_CACHE:
        _PROGRAM_CACHE[key] = _build_program(variant)
    nc = _PROGRAM_CACHE[key]
    res = run_bass_kernel_spmd(nc, in_maps, list(range(NCORES)))
    return res.results


def kernel(feats, W1, gamma1, beta1, W2, gamma2, beta2, neigh):
    feats = np.asarray(feats, np.float32)
    W1 = np.asarray(W1, np.float32)
    W2 = np.asarray(W2, np.float32)
    gamma1 = np.asarray(gamma1, np.float32)
    beta1 = np.asarray(beta1, np.float32)
    gamma2 = np.asarray(gamma2, np.float32)
    beta2 = np.asarray(beta2, np.float32)
    neigh_np = np.asarray(neigh)

    prep = None
    try:
        prep = _prepare_host(feats, W1, gamma1, beta1, W2, gamma2, beta2, neigh_np)
    except Exception:
        prep = None
    if prep is None:
        return _numpy_path(feats, W1, gamma1, beta1, W2, gamma2, beta2,
                           neigh_np.astype(np.int64))

    in_maps, own_ids = prep
    results = _run_device(in_maps)
    out = np.empty((N, C), np.float32)
    for c in range(NCORES):
        out[own_ids[c]] = results[c]["out"]
    return out



# revision 2
# speedup vs baseline: 1027.4561x; 1027.4561x over previous
"""MinkowskiResBlock on 8 TRN2 NeuronCores.

Strategy: spatially shard the N=131072 points across 8 cores (coords are
reconstructed from the labeled 27-offset neighbor graph), per-core local
gather tables (own shard + halo) in HBM, transposed dma_gather feeding bf16
matmuls that accumulate out^T in PSUM, BN stats via ACT accum + AllReduce,
halo exchange of the intermediate activations via export-gather + AllGather
+ parity import-gathers.  Falls back to a NumPy reference path if the
neighbor graph is not grid-consistent or a shard overflows its caps.
"""

import numpy as np
import ml_dtypes

N, C, K, NCORES = 131072, 192, 27, 8
S = N // NCORES            # 16384 points per core
ELEM = 256                 # bf16 elems per table row (C=192 + 64 pad) = 512B
RT = 512                   # rowtile (gather size / PSUM free dim)
NT = S // RT               # 32 rowtiles per core
HALO_CAP = 8192
EXP_CAP = 8192             # per-core export slots (8 * 8192 = 65536 rows)
IMP_CAP = 4096             # per parity class
ZROW1 = S + HALO_CAP       # T1 zero row
L1 = ZROW1 + 1
ZROW2 = S + 2 * IMP_CAP    # T2 zero row
L2 = ZROW2 + 1
BN_EPS = 1e-5

OFFS = np.array(
    [[dx, dy, dz] for dx in (-1, 0, 1) for dy in (-1, 0, 1) for dz in (-1, 0, 1)],
    np.int64,
)

_PROGRAM_CACHE = {}


# ----------------------------------------------------------------------------
# host-side graph analysis / sharding
# ----------------------------------------------------------------------------

def _spatial_order(neigh):
    """Reconstruct voxel coords from the labeled neighbor graph; return a
    spatial ordering of the N points, or None if the graph is inconsistent."""
    nb_all = neigh.astype(np.int64)
    if nb_all.shape != (K, N) or nb_all.min() < 0 or nb_all.max() > N:
        return None
    coords = np.zeros((N, 3), np.int64)
    comp = np.full(N, -1, np.int64)
    visited = np.zeros(N, bool)
    ncomp = 0
    while True:
        seeds = np.flatnonzero(~visited)
        if seeds.size == 0:
            break
        seed = seeds[0]
        visited[seed] = True
        comp[seed] = ncomp
        frontier = np.array([seed], np.int64)
        while frontier.size:
            new = []
            for k in range(K):
                if k == 13:
                    continue
                nb = nb_all[k][frontier]
                valid = nb < N
                if not valid.any():
                    continue
                src = frontier[valid]
                dst = nb[valid]
                fresh = ~visited[dst]
                if fresh.any():
                    d = dst[fresh]
                    s = src[fresh]
                    coords[d] = coords[s] + OFFS[k]
                    visited[d] = True
                    comp[d] = ncomp
                    new.append(d)
            frontier = (
                np.unique(np.concatenate(new)) if new else np.array([], np.int64)
            )
        ncomp += 1
        if ncomp > 64:  # clearly not a sparse voxel grid
            return None
    # validate every edge against its labeled offset
    for k in range(K):
        if k == 13:
            continue
        nb = nb_all[k]
        valid = np.flatnonzero(nb < N)
        if valid.size == 0:
            continue
        dst = nb[valid]
        if not (comp[dst] == comp[valid]).all():
            return None
        if not (coords[dst] == coords[valid] + OFFS[k]).all():
            return None
    if not (nb_all[13][nb_all[13] < N] == np.flatnonzero(nb_all[13] < N)).all():
        # self-offset must map each point to itself where valid
        pass  # not strictly required for correctness of our scheme
    key = coords - coords.min(axis=0)
    return np.lexsort((key[:, 2], key[:, 1], key[:, 0], comp))


def _pack_idx(flat):
    """int16 [n] (n % 16 == 0) -> [128, n//16] in the firmware layout:
    idx j at partition j%16, offset j//16, replicated across the 8
    16-partition groups (each SWDGE queue's Q7 pair reads its own group)."""
    lay = flat.reshape(-1, 16).T.astype(np.int16)  # [16, n//16]
    return np.tile(lay, (8, 1))


def _prepare_host(feats, W1, gamma1, beta1, W2, gamma2, beta2, neigh):
    order = _spatial_order(neigh)
    if order is None:
        return None
    nb = neigh.astype(np.int64)
    owner = np.empty(N, np.int64)
    ownpos = np.empty(N, np.int64)
    for c in range(NCORES):
        ids = order[c * S:(c + 1) * S]
        owner[ids] = c
        ownpos[ids] = np.arange(S)

    own_ids, halos = [], []
    for c in range(NCORES):
        ids = order[c * S:(c + 1) * S]
        own_ids.append(ids)
        fan = nb[:, ids].ravel()
        fan = np.unique(fan[fan < N])
        halo = fan[owner[fan] != c]
        if halo.size > HALO_CAP:
            return None
        halos.append(halo)

    # per-source export lists: union of halo rows each core must serve
    exp_ids = [[] for _ in range(NCORES)]
    for c in range(NCORES):
        for s, cnt in zip(*np.unique(owner[halos[c]], return_counts=True)):
            exp_ids[int(s)].append(halos[c][owner[halos[c]] == s])
    exports = []
    pos = np.full(N, -1, np.int64)  # global export-table position per id
    for s in range(NCORES):
        e = (
            np.unique(np.concatenate(exp_ids[s]))
            if exp_ids[s]
            else np.array([], np.int64)
        )
        if e.size > EXP_CAP:
            return None
        exports.append(e)
        pos[e] = s * EXP_CAP + np.arange(e.size)

    feats_bf = np.zeros((N + 1, ELEM), ml_dtypes.bfloat16)
    feats_bf[:N, :C] = feats.astype(ml_dtypes.bfloat16)

    w_scale = 1.0  # weights used as-is
    def pack_w(W):
        wp = np.zeros((K, 2, 128, C), ml_dtypes.bfloat16)
        for k in range(K):
            wp[k, 0, :, :] = (W[k][0:128, :] * w_scale).astype(ml_dtypes.bfloat16)
            wp[k, 1, 0:64, :] = (W[k][128:192, :] * w_scale).astype(ml_dtypes.bfloat16)
        return wp

    w1p, w2p = pack_w(W1), pack_w(W2)

    gb = np.zeros((128, 8), np.float32)
    gb[:, 0] = gamma1[0:128]
    gb[0:64, 1] = gamma1[128:192]
    gb[:, 2] = beta1[0:128]
    gb[0:64, 3] = beta1[128:192]
    gb[:, 4] = gamma2[0:128]
    gb[0:64, 5] = gamma2[128:192]
    gb[:, 6] = beta2[0:128]
    gb[0:64, 7] = beta2[128:192]

    eye16 = np.eye(128, dtype=ml_dtypes.bfloat16)
    eye32 = np.eye(128, dtype=np.float32)

    in_maps = []
    for c in range(NCORES):
        ids = own_ids[c]
        halo = halos[c]

        # T1: [own | halo(sorted) | zero]
        t1 = np.zeros((L1, ELEM), ml_dtypes.bfloat16)
        t1[0:S] = feats_bf[ids]
        t1[S:S + halo.size] = feats_bf[halo]
        loc1 = np.full(N + 1, ZROW1, np.int64)
        loc1[ids] = np.arange(S)
        loc1[halo] = S + np.arange(halo.size)

        # T2 layout: [own | imp_even | imp_odd | zero]
        hpos = pos[halo]
        assert (hpos >= 0).all()
        even_m = (hpos & 1) == 0
        he, ho = halo[even_m], halo[~even_m]
        if he.size > IMP_CAP or ho.size > IMP_CAP:
            return None
        loc2 = np.full(N + 1, ZROW2, np.int64)
        loc2[ids] = np.arange(S)
        loc2[he] = S + np.arange(he.size)
        loc2[ho] = S + IMP_CAP + np.arange(ho.size)

        nbo = nb[:, ids]  # [27, S] global
        idx1 = loc1[nbo]  # [27, S]
        idx2 = loc2[nbo]
        assert idx1.max() < 32768 and idx2.max() < 32768

        idx1_dev = np.zeros((128, K, NT, RT // 16), np.int16)
        idx2_dev = np.zeros((128, K, NT, RT // 16), np.int16)
        for k in range(K):
            for t in range(NT):
                idx1_dev[:, k, t, :] = _pack_idx(idx1[k, t * RT:(t + 1) * RT])
                idx2_dev[:, k, t, :] = _pack_idx(idx2[k, t * RT:(t + 1) * RT])

        # export gather: local own rows to publish (padded with 0)
        eloc = np.zeros(EXP_CAP, np.int64)
        eloc[:exports[c].size] = ownpos[exports[c]]
        exp_dev = np.zeros((128, EXP_CAP // RT, RT // 16), np.int16)
        for gidx in range(EXP_CAP // RT):
            exp_dev[:, gidx, :] = _pack_idx(eloc[gidx * RT:(gidx + 1) * RT])

        # import gathers: ag positions >> 1 per parity class (padded with 0)
        imp_dev = np.zeros((128, 2, IMP_CAP // RT, RT // 16), np.int16)
        for pcls, h in enumerate((he, ho)):
            ip = np.zeros(IMP_CAP, np.int64)
            ip[:h.size] = pos[h] >> 1
            for gidx in range(IMP_CAP // RT):
                imp_dev[:, pcls, gidx, :] = _pack_idx(ip[gidx * RT:(gidx + 1) * RT])

        featsT = np.zeros((2, 128, S), np.float32)
        fo = feats[ids]  # [S, C] fp32
        featsT[0] = fo[:, 0:128].T
        featsT[1, 0:64] = fo[:, 128:192].T

        in_maps.append({
            "t1": t1,
            "idx1": idx1_dev,
            "idx2": idx2_dev,
            "expidx": exp_dev,
            "impidx": imp_dev,
            "w1": w1p,
            "w2": w2p,
            "gb": gb,
            "eye16": eye16,
            "eye32": eye32,
            "featsT": featsT,
        })
    return in_maps, own_ids


# ----------------------------------------------------------------------------
# device program
# ----------------------------------------------------------------------------

def _build_program(variant="full"):
    import concourse.tile as tile
    from concourse import bacc, mybir

    AF = mybir.ActivationFunctionType
    ALU = mybir.AluOpType
    bf16, f32, i16 = mybir.dt.bfloat16, mybir.dt.float32, mybir.dt.int16

    nc = bacc.Bacc("TRN2", target_bir_lowering=False, debug=False,
                   num_devices=NCORES, num_swdge_queues=4)
    if variant == "floor":
        out_f = nc.dram_tensor("out", [S, C], f32, kind="ExternalOutput")
        with tile.TileContext(nc) as tc:
            with tc.tile_pool(name="fl", bufs=1) as flp:
                z = flp.tile([128, 16, C], f32, tag="z")
                nc.vector.memset(z[:], 0.0)
                for j in range(8):
                    nc.sync.dma_start(
                        out_f[j * 2048:(j + 1) * 2048].rearrange(
                            "(t p) e -> p t e", p=128), z[:])
        nc.compile()
        return nc

    t1_d = nc.dram_tensor("t1", [L1, ELEM], bf16, kind="ExternalInput")
    idx1_d = nc.dram_tensor("idx1", [128, K, NT, RT // 16], i16, kind="ExternalInput")
    idx2_d = nc.dram_tensor("idx2", [128, K, NT, RT // 16], i16, kind="ExternalInput")
    exp_d = nc.dram_tensor("expidx", [128, EXP_CAP // RT, RT // 16], i16, kind="ExternalInput")
    imp_d = nc.dram_tensor("impidx", [128, 2, IMP_CAP // RT, RT // 16], i16, kind="ExternalInput")
    w1_d = nc.dram_tensor("w1", [K, 2, 128, C], bf16, kind="ExternalInput")
    w2_d = nc.dram_tensor("w2", [K, 2, 128, C], bf16, kind="ExternalInput")
    gb_d = nc.dram_tensor("gb", [128, 8], f32, kind="ExternalInput")
    eye16_d = nc.dram_tensor("eye16", [128, 128], bf16, kind="ExternalInput")
    eye32_d = nc.dram_tensor("eye32", [128, 128], f32, kind="ExternalInput")
    fT_d = nc.dram_tensor("featsT", [2, 128, S], f32, kind="ExternalInput")
    out_d = nc.dram_tensor("out", [S, C], f32, kind="ExternalOutput")

    with tile.TileContext(nc) as tc:
        with (
            tc.tile_pool(name="const", bufs=1) as constp,
            tc.tile_pool(name="widx", bufs=1) as widxp,
            tc.tile_pool(name="idxs", bufs=2) as idxp,
            tc.tile_pool(name="big", bufs=1) as bigp,
            tc.tile_pool(name="gat", bufs=6) as gatp,
            tc.tile_pool(name="work", bufs=2) as workp,
            tc.tile_pool(name="stat", bufs=1) as statp,
            tc.tile_pool(name="pacc", bufs=2, space="PSUM") as paccp,
            tc.tile_pool(name="ptr", bufs=2, space="PSUM") as ptrp,
            tc.tile_pool(name="dram", bufs=1, space="DRAM") as dramp,
        ):
            # ---------------- constants ----------------
            gb_t = constp.tile([128, 8], f32)
            nc.sync.dma_start(gb_t[:], gb_d[:])
            eye16_t = constp.tile([128, 128], bf16)
            nc.sync.dma_start(eye16_t[:], eye16_d[:])
            eye32_t = constp.tile([128, 128], f32)
            nc.sync.dma_start(eye32_t[:], eye32_d[:])
            expidx_t = constp.tile([128, EXP_CAP // RT, RT // 16], i16)
            nc.sync.dma_start(expidx_t[:], exp_d[:])
            impidx_t = constp.tile([128, 2, IMP_CAP // RT, RT // 16], i16)
            nc.sync.dma_start(impidx_t[:], imp_d[:])

            # internal DRAM
            t2_t = dramp.tile([L2, ELEM], bf16)
            expbuf_t = dramp.tile([EXP_CAP, ELEM], bf16)
            ag_t = dramp.tile([NCORES * EXP_CAP, ELEM], bf16, addr_space="Shared")
            ar_in = [dramp.tile([128, 4], f32, name=f"ar_in{i}") for i in range(2)]
            ar_out = [dramp.tile([128, 4], f32, addr_space="Shared", name=f"ar_out{i}")
                      for i in range(2)]

            # persistent per-conv state (tags shared between convs)
            o_a = [None]
            o_b = [None]

            do_gather = variant != "mm"
            do_mm = variant not in ("gath", "gath4q")
            gq = (lambda k: k % 4) if variant == "gath4q" else (lambda k: 0)

            def conv(conv_i, table_ap, idx_dram, w_tile):
                """Gather-GEMM over 27 offsets; returns nothing, fills
                o_a/o_b (bf16 out^T halves) and stats tiles."""
                sums = statp.tile([128, NT, 2], f32, tag=f"sums{conv_i}")
                sums_b = statp.tile([64, NT, 2], f32, tag=f"sumsb{conv_i}")
                oa = bigp.tile([128, S], bf16, tag="o_a")
                ob = bigp.tile([64, S], bf16, tag="o_b")
                if not do_mm:
                    nc.vector.memset(sums[:], 0.0)
                    nc.vector.memset(sums_b[:], 0.0)
                    nc.vector.memset(oa[:], 0.0)
                    nc.vector.memset(ob[:], 0.0)
                o_a[0], o_b[0] = oa, ob
                IG = 4  # rowtiles per idx load
                for t in range(NT):
                    if t % IG == 0:
                        idxg = idxp.tile([128, K, IG, RT // 16], i16, tag="idxg")
                        nc.sync.dma_start(idxg[:], idx_dram[:, :, t:t + IG, :])
                    p1 = paccp.tile([128, RT], f32, tag="p1")
                    p2 = paccp.tile([64, RT], f32, tag="p2")
                    for k in range(K):
                        if do_gather or (k == 0 and t == 0):
                            g = gatp.tile([128, 2, RT], bf16, tag="g")
                            nc.gpsimd.dma_gather(
                                g[:], table_ap, idxg[:, k, t % IG, :], RT, RT, ELEM,
                                transpose=True, queue_num=gq(k),
                            )
                            glast = g
                        else:
                            g = glast
                        if do_mm:
                            for r in range(2):
                                nc.tensor.matmul(
                                    p1[:], w_tile[:, k, r, 0:128], g[:, r, :],
                                    start=(k == 0 and r == 0),
                                    stop=(k == K - 1 and r == 1),
                                )
                            for r in range(2):
                                nc.tensor.matmul(
                                    p2[:], w_tile[:, k, r, 128:192], g[:, r, :],
                                    start=(k == 0 and r == 0),
                                    stop=(k == K - 1 and r == 1),
                                )
                    if not do_mm:
                        continue
                    ts = slice(t * RT, (t + 1) * RT)
                    sq1 = workp.tile([128, RT], f32, tag="sq1")
                    sq2 = workp.tile([64, RT], f32, tag="sq2")
                    nc.scalar.activation(oa[:, ts], p1[:], AF.Copy,
                                         accum_out=sums[:, t, 0:1])
                    nc.scalar.activation(sq1[:], p1[:], AF.Square,
                                         accum_out=sums[:, t, 1:2])
                    nc.scalar.activation(ob[:, ts], p2[:], AF.Copy,
                                         accum_out=sums_b[:, t, 0:1])
                    nc.scalar.activation(sq2[:], p2[:], AF.Square,
                                         accum_out=sums_b[:, t, 1:2])
                return sums, sums_b

            def bn_coeffs(conv_i, sums, sums_b):
                """AllReduce the per-rowtile partial sums; compute per-channel
                a = gamma*rsqrt(var+eps), b = beta - mean*a as [128,1]/[64,1]."""
                tot = statp.tile([128, 4], f32, tag=f"tot{conv_i}")
                junk1 = workp.tile([128, NT], f32, tag="sq1")
                junk2 = workp.tile([64, NT], f32, tag="sq2")
                nc.vector.memset(tot[:], 0.0)
                nc.scalar.activation(junk1[:], sums[:, :, 0], AF.Copy,
                                     accum_out=tot[:, 0:1])
                nc.scalar.activation(junk1[:], sums[:, :, 1], AF.Copy,
                                     accum_out=tot[:, 1:2])
                nc.scalar.activation(junk2[:], sums_b[:, :, 0], AF.Copy,
                                     accum_out=tot[0:64, 2:3])
                nc.scalar.activation(junk2[:], sums_b[:, :, 1], AF.Copy,
                                     accum_out=tot[0:64, 3:4])
                nc.sync.dma_start(ar_in[conv_i][:], tot[:])
                rtot = statp.tile([128, 4], f32, tag=f"rtot{conv_i}")
                if variant == "noar":
                    nc.vector.tensor_scalar_mul(rtot[:], tot[:], float(NCORES))
                else:
                    nc.gpsimd.collective_compute(
                        "AllReduce", ALU.add,
                        replica_groups=[list(range(NCORES))],
                        ins=[ar_in[conv_i].opt()], outs=[ar_out[conv_i].opt()],
                    )
                    nc.sync.dma_start(rtot[:], ar_out[conv_i][:])
                # mean/var/a/b per partition, lo (cols 0:2) and hi (cols 2:4)
                co = statp.tile([128, 8], f32, tag=f"co{conv_i}")
                ga = gb_t[:, 4 * conv_i + 0:4 * conv_i + 1]
                ga_h = gb_t[0:64, 4 * conv_i + 1:4 * conv_i + 2]
                be = gb_t[:, 4 * conv_i + 2:4 * conv_i + 3]
                be_h = gb_t[0:64, 4 * conv_i + 3:4 * conv_i + 4]
                invn = 1.0 / float(N)
                for half, (sm, sq, gg, bb) in enumerate((
                    (rtot[:, 0:1], rtot[:, 1:2], ga, be),
                    (rtot[0:64, 2:3], rtot[0:64, 3:4], ga_h, be_h),
                )):
                    p = slice(0, 128) if half == 0 else slice(0, 64)
                    mean = co[p, 4 * half + 0:4 * half + 1]
                    var = co[p, 4 * half + 1:4 * half + 2]
                    a = co[p, 4 * half + 2:4 * half + 3]
                    b = co[p, 4 * half + 3:4 * half + 4]
                    nc.vector.tensor_scalar_mul(mean, sm, invn)
                    nc.vector.tensor_scalar_mul(var, sq, invn)
                    # var -= mean^2 ; var += eps
                    nc.vector.tensor_tensor(a, mean, mean, ALU.mult)
                    nc.vector.tensor_tensor(var, var, a, ALU.subtract)
                    nc.vector.tensor_scalar_add(var, var, BN_EPS)
                    nc.scalar.sqrt(a, var)
                    nc.vector.reciprocal(a, a)          # a = rsqrt(var+eps)
                    nc.vector.tensor_tensor(a, a, gg, ALU.mult)
                    nc.vector.tensor_tensor(b, mean, a, ALU.mult)
                    nc.vector.tensor_tensor(b, bb, b, ALU.subtract)
                return co

            # ======================= conv1 =======================
            w1_t = widxp.tile([128, K, 2, C], bf16, tag="w")
            for k in range(K):
                for r in range(2):
                    nc.sync.dma_start(w1_t[:, k, r, :], w1_d[k, r])

            sums1, sums1_b = conv(0, t1_d[:, :], idx1_d, w1_t)
            if variant == "nobn":
                with tc.tile_pool(name="dumm", bufs=2) as dummp:
                    for t in range(NT):
                        z2 = dummp.tile([128, 4, C], f32, tag="z2")
                        nc.vector.tensor_copy(z2[:, 0, 0:1], sums1[:, 0, 0:1])
                        nc.vector.memset(z2[:, 1:, :], 0.0)
                        nc.sync.dma_start(
                            out_d[t * RT:(t + 1) * RT].rearrange(
                                "(rb p) e -> p rb e", p=128), z2[:])
                nc.compile()
                return nc
            co1 = bn_coeffs(0, sums1, sums1_b)
            a1 = co1[:, 2:3]
            b1 = co1[:, 3:4]
            a1h = co1[0:64, 6:7]
            b1h = co1[0:64, 7:8]

            # BN1 + relu -> h (bf16), transpose to row-major, write T2 own
            oa, ob = o_a[0], o_b[0]
            for t in range(NT):
                ts = slice(t * RT, (t + 1) * RT)
                h1 = workp.tile([128, RT], bf16, tag="h1")
                h2 = workp.tile([64, RT], bf16, tag="h2")
                nc.scalar.activation(h1[:], oa[:, ts], AF.Relu, bias=b1, scale=a1)
                nc.scalar.activation(h2[:], ob[:, ts], AF.Relu, bias=b1h, scale=a1h)
                stage = workp.tile([128, 4, ELEM], bf16, tag="stage")
                nc.vector.memset(stage[:, :, 192:256], 0.0)
                for rb in range(4):
                    tr1 = ptrp.tile([128, 128], bf16, tag="tr1")
                    nc.tensor.transpose(tr1[:], h1[:, rb * 128:(rb + 1) * 128],
                                        eye16_t[:])
                    tr2 = ptrp.tile([128, 64], bf16, tag="tr2")
                    nc.tensor.transpose(tr2[:], h2[:, rb * 128:(rb + 1) * 128],
                                        eye16_t[0:64, 0:64])
                    nc.vector.tensor_copy(stage[:, rb, 0:128], tr1[:])
                    nc.vector.tensor_copy(stage[:, rb, 128:192], tr2[:])
                nc.sync.dma_start(
                    t2_t[ts].rearrange("(rb p) e -> p rb e", p=128), stage[:]
                )
            # T2 zero row
            zrow = constp.tile([1, ELEM], bf16)
            nc.vector.memset(zrow[:], 0.0)
            nc.sync.dma_start(t2_t[ZROW2:ZROW2 + 1, :], zrow[:])

            if variant in ("conv1", "gath", "mm", "gath4q", "noar"):
                for t in range(NT):
                    z = workp.tile([128, 4, C], f32, tag="ostage")
                    nc.vector.memset(z[:], 0.0)
                    nc.sync.dma_start(
                        out_d[t * RT:(t + 1) * RT].rearrange(
                            "(rb p) e -> p rb e", p=128), z[:])
            do_rest = variant == "full"
            if do_rest:
    # ---------------- halo exchange ----------------
                for gidx in range(EXP_CAP // RT):
                    ge = gatp.tile([128, RT // 128, ELEM], bf16, tag="ge")
                    nc.gpsimd.dma_gather(
                        ge[:], t2_t[0:S, :], expidx_t[:, gidx, :], RT, RT, ELEM,
                        transpose=False, queue_num=1 + gidx % 3,
                    )
                    nc.sync.dma_start(
                        expbuf_t[gidx * RT:(gidx + 1) * RT].rearrange(
                            "(t p) e -> p t e", p=128),
                        ge[:],
                    )
                nc.gpsimd.collective_compute(
                    "AllGather", mybir.AluOpType.bypass,
                    replica_groups=[list(range(NCORES))],
                    ins=[expbuf_t.opt()], outs=[ag_t.opt()],
                )
                ag_pairs = ag_t[:].rearrange("(a b) e -> a (b e)", b=2)  # [32768, 512]
                for pcls in range(2):
                    src = ag_pairs[:, pcls * ELEM:(pcls + 1) * ELEM]
                    for gidx in range(IMP_CAP // RT):
                        gi = gatp.tile([128, RT // 128, ELEM], bf16, tag="ge")
                        nc.gpsimd.dma_gather(
                            gi[:], src, impidx_t[:, pcls, gidx, :], RT, RT, ELEM,
                            elem_step=2 * ELEM,
                            transpose=False, queue_num=1 + gidx % 3,
                        )
                        base = S + pcls * IMP_CAP + gidx * RT
                        nc.sync.dma_start(
                            t2_t[base:base + RT].rearrange("(t p) e -> p t e", p=128),
                            gi[:],
                        )

                # ======================= conv2 =======================
                w2_t = widxp.tile([128, K, 2, C], bf16, tag="w")
                for k in range(K):
                    for r in range(2):
                        nc.sync.dma_start(w2_t[:, k, r, :], w2_d[k, r])

                sums2, sums2_b = conv(1, t2_t[:, :], idx2_d, w2_t)
                co2 = bn_coeffs(1, sums2, sums2_b)
                a2 = co2[:, 2:3]
                b2 = co2[:, 3:4]
                a2h = co2[0:64, 6:7]
                b2h = co2[0:64, 7:8]

                # BN2 + residual + relu -> transpose -> out
                oa, ob = o_a[0], o_b[0]
                for t in range(NT):
                    ts = slice(t * RT, (t + 1) * RT)
                    tmp1 = workp.tile([128, RT], f32, tag="tmp1")
                    tmp2 = workp.tile([64, RT], f32, tag="tmp2")
                    nc.vector.tensor_scalar(tmp1[:], oa[:, ts], a2, b2,
                                            ALU.mult, ALU.add)
                    nc.vector.tensor_scalar(tmp2[:], ob[:, ts], a2h, b2h,
                                            ALU.mult, ALU.add)
                    ft1 = workp.tile([128, RT], f32, tag="ft1")
                    nc.sync.dma_start(ft1[:], fT_d[0, :, ts])
                    ft2 = workp.tile([64, RT], f32, tag="ft2")
                    nc.sync.dma_start(ft2[:], fT_d[1, 0:64, ts])
                    nc.vector.tensor_add(tmp1[:], tmp1[:], ft1[:])
                    nc.vector.tensor_add(tmp2[:], tmp2[:], ft2[:])
                    nc.vector.tensor_scalar_max(tmp1[:], tmp1[:], 0.0)
                    nc.vector.tensor_scalar_max(tmp2[:], tmp2[:], 0.0)
                    ostage = workp.tile([128, 4, C], f32, tag="ostage")
                    for rb in range(4):
                        tr1 = ptrp.tile([128, 128], f32, tag="tr1")
                        nc.tensor.transpose(tr1[:], tmp1[:, rb * 128:(rb + 1) * 128],
                                            eye32_t[:])
                        tr2 = ptrp.tile([128, 64], f32, tag="tr2")
                        nc.tensor.transpose(tr2[:], tmp2[:, rb * 128:(rb + 1) * 128],
                                            eye32_t[0:64, 0:64])
                        nc.vector.tensor_copy(ostage[:, rb, 0:128], tr1[:])
                        nc.vector.tensor_copy(ostage[:, rb, 128:192], tr2[:])
                    nc.sync.dma_start(
                        out_d[ts].rearrange("(rb p) e -> p rb e", p=128), ostage[:]
                    )

    nc.compile()
    return nc


# ----------------------------------------------------------------------------
# numpy fallback (also the correctness oracle for arbitrary inputs)
# ----------------------------------------------------------------------------

def _numpy_path(feats, W1, gamma1, beta1, W2, gamma2, beta2, neigh):
    def conv(f, W):
        pad = np.concatenate([f, np.zeros((1, f.shape[1]), f.dtype)], axis=0)
        out = np.zeros_like(f)
        for k in range(W.shape[0]):
            out += pad[neigh[k]] @ W[k]
        return out

    def bn(x, g, b):
        m = x.mean(axis=0)
        v = x.var(axis=0)
        return (x - m) / np.sqrt(v + BN_EPS) * g + b

    out = conv(feats, W1)
    out = np.maximum(bn(out, gamma1, beta1), 0)
    out = conv(out, W2)
    out = bn(out, gamma2, beta2)
    return np.maximum(out + feats, 0).astype(np.float32)


# ----------------------------------------------------------------------------
# compiled executor: trace/compile once, keep inputs device-resident
# ----------------------------------------------------------------------------

_EXEC_CACHE = {}


class _Exec:
    """Wraps one Bass program variant as a persistently-compiled 8-core jax
    callable.  Unlike run_bass_kernel_spmd (which re-traces and re-uploads
    every call), the jit wrapper and staged device inputs are cached, so a
    steady-state call is a single async dispatch of the prebuilt NEFF."""

    def __init__(self, variant="full"):
        import jax
        from jax.experimental.shard_map import shard_map
        from jax.sharding import Mesh, NamedSharding, PartitionSpec
        from concourse import mybir
        from concourse.bass2jax import (
            _bass_exec_p, install_neuronx_cc_hook, partition_id_tensor)

        install_neuronx_cc_hook()
        key = f"nc:{variant}"
        if key not in _PROGRAM_CACHE:
            _PROGRAM_CACHE[key] = _build_program(variant)
        nc = _PROGRAM_CACHE[key]
        self._jax = jax
        self._np = np

        partition_name = (nc.partition_id_tensor.name
                          if nc.partition_id_tensor else None)
        in_names, out_names, out_avals = [], [], []
        for alloc in nc.m.functions[0].allocations:
            if not isinstance(alloc, mybir.MemoryLocationSet):
                continue
            name = alloc.memorylocations[0].name
            if alloc.kind == "ExternalInput":
                if name != partition_name:
                    in_names.append(name)
            elif alloc.kind == "ExternalOutput":
                out_names.append(name)
                out_avals.append(jax.core.ShapedArray(
                    tuple(alloc.tensor_shape), mybir.dt.np(alloc.dtype)))
        self.in_names = in_names
        self.out_names = out_names
        self.out_avals = out_avals
        n_params = len(in_names)
        bind_names = tuple(in_names + out_names
                           + ([partition_name] if partition_name else []))

        def _body(*args):
            operands = list(args)
            if partition_name is not None:
                operands.append(partition_id_tensor())
            return tuple(_bass_exec_p.bind(
                *operands,
                out_avals=tuple(out_avals),
                in_names=bind_names,
                out_names=tuple(out_names),
                lowering_input_output_aliases=(),
                sim_require_finite=True,
                sim_require_nnan=True,
                nc=nc,
            ))

        devices = jax.devices()[:NCORES]
        assert len(devices) == NCORES
        self.mesh = Mesh(np.asarray(devices), ("core",))
        n_args = n_params + len(out_names)
        self._fn = jax.jit(
            shard_map(_body, mesh=self.mesh,
                      in_specs=(PartitionSpec("core"),) * n_args,
                      out_specs=(PartitionSpec("core"),) * len(out_names),
                      check_rep=False),
            keep_unused=True,
        )
        self._sharding = NamedSharding(self.mesh, PartitionSpec("core"))
        # out buffers: the program fully writes every ExternalOutput element,
        # so non-donated (reusable) zero inputs are sufficient.
        self._zeros = [
            jax.device_put(
                np.zeros((NCORES * a.shape[0], *a.shape[1:]), a.dtype),
                self._sharding)
            for a in out_avals
        ]
        self._staged = {}

    def stage(self, in_maps):
        """Concat per-core inputs and device_put once; cached by identity."""
        key = id(in_maps)
        hit = self._staged.get(key)
        if hit is not None:
            return hit
        jax = self._jax
        concat = [
            np.concatenate([np.asarray(in_maps[c][n]) for c in range(NCORES)],
                           axis=0)
            for n in self.in_names
        ]
        staged = [jax.device_put(a, self._sharding) for a in concat]
        for a in staged:
            a.block_until_ready()
        self._staged = {key: staged}  # keep only the latest staging
        return staged

    def run(self, staged):
        """One async dispatch; returns unfetched jax output arrays."""
        return self._fn(*staged, *self._zeros)

    def fetch(self, outs):
        """Fetch outputs to host as per-core dicts."""
        return [
            {n: np.asarray(outs[i]).reshape(
                NCORES, *self.out_avals[i].shape)[c]
             for i, n in enumerate(self.out_names)}
            for c in range(NCORES)
        ]


def _get_exec(variant="full"):
    if variant not in _EXEC_CACHE:
        _EXEC_CACHE[variant] = _Exec(variant)
    return _EXEC_CACHE[variant]


# ----------------------------------------------------------------------------
# entry point
# ----------------------------------------------------------------------------

def _run_device(in_maps, variant="full"):
    ex = _get_exec(variant)
    staged = ex.stage(in_maps)
    outs = ex.run(staged)
    return ex.fetch(outs)


def kernel(feats, W1, gamma1, beta1, W2, gamma2, beta2, neigh):
    feats = np.asarray(feats, np.float32)
    W1 = np.asarray(W1, np.float32)
    W2 = np.asarray(W2, np.float32)
    gamma1 = np.asarray(gamma1, np.float32)
    beta1 = np.asarray(beta1, np.float32)
    gamma2 = np.asarray(gamma2, np.float32)
    beta2 = np.asarray(beta2, np.float32)
    neigh_np = np.asarray(neigh)

    prep = None
    try:
        prep = _prepare_host(feats, W1, gamma1, beta1, W2, gamma2, beta2, neigh_np)
    except Exception:
        prep = None
    if prep is None:
        return _numpy_path(feats, W1, gamma1, beta1, W2, gamma2, beta2,
                           neigh_np.astype(np.int64))

    in_maps, own_ids = prep
    results = _run_device(in_maps)
    out = np.empty((N, C), np.float32)
    for c in range(NCORES):
        out[own_ids[c]] = results[c]["out"]
    return out

